# revision 9
# baseline (speedup 1.0000x reference)
"""DeltaSynapse kernel for Trainium2 (8 NeuronCores, SPMD).

Reference computation:
    Xpre[b,e,o] = sum_d delaymap[d,e,o] * Xd[d,b,e]
    I[b,o]      = sum_e (signs*W)[e,o] * Xpre[b,e,o]

Folded:  I[b,o] = sum_{d,e} (delaymap[d,e,o] * Weff[e,o]) * Xd[d,b,e]
i.e. a sum of D matmuls  I += Xd[d] @ (delaymap[d] . Weff).

Sharding: column-shard the post dim `o` across 8 cores (256 cols each).
Each core reads its own slice of delaymap/W/signs plus a replicated Xd
(~21 MiB/core) and writes a disjoint [16, 256] output slice -> host
concat. Memory-bound: per-core roofline ~ 21 MiB / ~358 GB/s ~ 60 us.
"""

import numpy as np

D, B, N = 8, 16, 2048
NCORES = 8
P = 128                 # SBUF partitions / matmul contraction tile
OSH = N // NCORES       # per-core post-dim shard = 256
NCH = N // P            # e-chunks = 16
CGRP = 2                # e-chunks per DMA slab (2 MiB slabs)
NSLAB = NCH // CGRP

_prog_cache = {}


def _build_program():
    from concourse import bacc, tile
    from concourse import mybir

    f32 = mybir.dt.float32
    f32r = mybir.dt.float32r

    nc = bacc.Bacc()
    # Host-prepared layouts (see kernel() below):
    #   dm : [NSLAB, P, CGRP, D, OSH]   delaymap slice, e=(slab*CGRP+c2)*128+p
    #   wm : [P, NCH, OSH]              W slice,     e = c*128+p
    #   sm : [P, NCH, OSH]              signs slice
    #   xd : [P, NCH, D, B]             Xd transposed (replicated)
    dm = nc.dram_tensor("dm", [NSLAB, P, CGRP, D, OSH], f32, kind="ExternalInput")
    ws = nc.dram_tensor("ws", [P, 2, NCH, OSH], f32, kind="ExternalInput")
    xd = nc.dram_tensor("xd", [P, NCH, D, B], f32, kind="ExternalInput")
    out = nc.dram_tensor("out", [B, OSH], f32, kind="ExternalOutput")

    with tile.TileContext(nc) as tc:
        with (
            tc.tile_pool(name="const", bufs=1) as cpool,
            tc.tile_pool(name="dm", bufs=3) as dmpool,
            tc.tile_pool(name="wd", bufs=3) as wdpool,
            tc.tile_pool(name="psum", bufs=1, space="PSUM") as ppool,
            tc.tile_pool(name="outp", bufs=1) as opool,
        ):
            ws_t = cpool.tile([P, 2, NCH, OSH], f32)
            weff = cpool.tile([P, NCH, OSH], f32)
            xd_t = cpool.tile([P, NCH, D, B], f32)
            xd_r = cpool.tile([P, NCH, D, B], f32r)
            nc.sync.dma_start(ws_t[:], ws[:])
            nc.sync.dma_start(xd_t[:], xd[:])
            nc.vector.tensor_copy(xd_r[:], xd_t[:])
            nc.vector.tensor_mul(weff[:], ws_t[:, 0], ws_t[:, 1])

            psum = ppool.tile([B, OSH], f32)
            n_mm = NCH * D
            i = 0
            for slab in range(NSLAB):
                dm_t = dmpool.tile([P, CGRP, D, OSH], f32)
                nc.sync.dma_start(dm_t[:], dm[slab])
                wd_t = wdpool.tile([P, CGRP, D, OSH], f32r)
                for c2 in range(CGRP):
                    c = slab * CGRP + c2
                    weff_b = weff[:, c, :].unsqueeze(1).broadcast_to([P, D, OSH])
                    nc.vector.tensor_mul(wd_t[:, c2], dm_t[:, c2], weff_b)
                for c2 in range(CGRP):
                    c = slab * CGRP + c2
                    for d in range(D):
                        nc.tensor.matmul(
                            psum[:],
                            xd_r[:, c, d, :],
                            wd_t[:, c2, d, :],
                            start=(i == 0),
                            stop=(i == n_mm - 1),
                        )
                        i += 1

            out_t = opool.tile([B, OSH], f32)
            nc.scalar.copy(out_t[:], psum[:])
            nc.sync.dma_start(out[:], out_t[:])

    nc.compile()
    return nc


def _get_program():
    if "nc" not in _prog_cache:
        _prog_cache["nc"] = _build_program()
    return _prog_cache["nc"]


def _shard_inputs(Xd, delaymap, W, signs):
    """Pure layout permutation/slicing -> per-core input maps."""
    Xd = np.ascontiguousarray(np.asarray(Xd, dtype=np.float32))
    delaymap = np.asarray(delaymap, dtype=np.float32)
    W = np.asarray(W, dtype=np.float32)
    signs = np.asarray(signs, dtype=np.float32)

    # Xd [D,B,N] -> [P, NCH, D, B] (replicated to every core)
    xdT = np.ascontiguousarray(Xd.reshape(D, B, NCH, P).transpose(3, 2, 0, 1))

    in_maps = []
    for k in range(NCORES):
        osl = slice(k * OSH, (k + 1) * OSH)
        # delaymap [D,N,OSH] -> [NSLAB, P, CGRP, D, OSH]
        dmk = np.ascontiguousarray(
            delaymap[:, :, osl]
            .reshape(D, NSLAB, CGRP, P, OSH)
            .transpose(1, 3, 2, 0, 4)
        )
        wk = W[:, osl].reshape(NCH, P, OSH).transpose(1, 0, 2)
        sk = signs[:, osl].reshape(NCH, P, OSH).transpose(1, 0, 2)
        wsk = np.ascontiguousarray(np.stack([wk, sk], axis=1))
        in_maps.append({"dm": dmk, "ws": wsk, "xd": xdT})
    return in_maps


def _run(in_maps, trace=False, **kw):
    from concourse.bass_utils import run_bass_kernel_spmd

    nc = _get_program()
    return run_bass_kernel_spmd(nc, in_maps, list(range(NCORES)), trace=trace, **kw)


def kernel(Xd, X, delaymap, W, signs):
    in_maps = _shard_inputs(Xd, delaymap, W, signs)
    res = _run(in_maps)
    return np.concatenate(
        [res.results[k]["out"] for k in range(NCORES)], axis=1
    )
